# revision 25
# baseline (speedup 1.0000x reference)
"""LocalWindowAttention TRN2 kernel.

Full inputs -> full output. Sharding: 8 cores = batch(4) x seq-half(2).
Each core computes 2048 query positions; k/v halos (128 each side) come
from overlapping the per-core x slice, so no collectives are needed.

Math (per core, matching reference):
  qkv = x @ Wqkv + bqkv  (q pre-scaled by 1/sqrt(1024) via the store scale)
  banded attention, window 128, block size 128: query tile e attends key
  tiles e-1, e, e+1 with a static band mask |kpos - qpos| <= 128.
  Softmax without max-subtraction (scores are O(0.1)); invalid keys are
  zeroed AFTER exp via a 0/1 band mask (only the halo chunks 0/2 need
  masking; the center chunk is always fully in-band), out-of-sequence
  keys are zeroed via a validity indicator carried as a 65th column of v
  (which also yields the softmax denominator through the PV matmul).
  out = attn_out @ Wout + bout

Dtypes: q/k projection runs in fp8e4m3 DoubleRow (2x PE rate); inputs are
pre-scaled on the host (x*16, Wqk*64) to avoid the e4m3 subnormal range and
the scale is divided back out in the PSUM->SBUF store. The fp8 error only
perturbs attention scores (softmax attenuates it); v / PV / out-proj stay
fp16 since their quantization error would hit the output 1:1.
"""

import sys

import numpy as np

for _p in ("/opt/trn_rl_repo",):
    if _p not in sys.path:
        sys.path.insert(0, _p)

import ml_dtypes  # noqa: E402

import concourse.bass as bass  # noqa: E402,F401
import concourse.mybir as mybir  # noqa: E402
import concourse.tile as tile  # noqa: E402
from concourse import bacc  # noqa: E402
from concourse.bass_utils import run_bass_kernel_spmd  # noqa: E402
from concourse.masks import make_identity  # noqa: E402

F32 = mybir.dt.float32
FP16 = mybir.dt.float16
FP8 = mybir.dt.float8e4

B, S, D = 4, 4096, 1024
H, DH, W = 16, 64, 128
N_CORES = 8
S_LOC = 2048            # query positions per core
T_Q = S_LOC // W        # 16 query tiles per core
T_EXT = T_Q + 2         # 18 extended tiles (with halo)
S_EXT = T_EXT * W       # 2304
NQK = 2 * D             # q+k projected features
KC = D // 128           # 8 contraction chunks
KC2 = KC // 2           # 4 double-row chunks
HP = H // 2             # 8 head pairs
VCOL = DH + 1           # 64 v dims + indicator column

SIXTH = S_EXT // 6      # 384 positions per xT streaming chunk

X_SCALE = 16.0          # host pre-scale on x for fp8 (avoid subnormals)
W_SCALE = 64.0          # host pre-scale on Wqk for fp8
QK_DESCALE = 1.0 / (X_SCALE * W_SCALE)
Q_SCALE = QK_DESCALE / np.sqrt(D)   # extra 1/sqrt(d_model) on q


def _build_nc():
    nc = bacc.Bacc(
        "TRN2",
        target_bir_lowering=False,
        debug=False,
        num_devices=N_CORES,
    )

    xT_d = nc.dram_tensor("xT", [D, S_EXT], FP16, kind="ExternalInput").ap()
    xT8_d = nc.dram_tensor("xT8", [D, S_EXT], FP8, kind="ExternalInput").ap()
    # weights arrive pre-shuffled as (piece, kc, 128, 512) so every DMA
    # below reads one fully contiguous 64KB/128KB block
    wqk8_d = nc.dram_tensor("wqk8", [4 * KC * 128, 512], FP8, kind="ExternalInput").ap()
    wv_d = nc.dram_tensor("wv", [2 * KC * 128, 512], FP16, kind="ExternalInput").ap()
    bqk_d = nc.dram_tensor("bqk", [128, 16], F32, kind="ExternalInput").ap()
    bvb_d = nc.dram_tensor("bvb", [1, D], FP16, kind="ExternalInput").ap()
    wout_d = nc.dram_tensor("wout", [D, D], FP16, kind="ExternalInput").ap()
    boutb_d = nc.dram_tensor("boutb", [1, D], FP16, kind="ExternalInput").ap()
    trimask_d = nc.dram_tensor("trimask", [128, 2 * W], FP16, kind="ExternalInput").ap()
    indp_d = nc.dram_tensor("indp", [T_EXT, 128], F32, kind="ExternalInput").ap()
    out_d = nc.dram_tensor("out", [S_LOC, D], F32, kind="ExternalOutput").ap()

    with tile.TileContext(nc) as tc:
        _emit(tc, xT_d, xT8_d, wqk8_d, wv_d, bqk_d, bvb_d, wout_d, boutb_d,
              trimask_d, indp_d, out_d)
    nc.compile()
    return nc


def _emit(tc, xT_d, xT8_d, wqk8_d, wv_d, bqk_d, bvb_d, wout_d, boutb_d,
          trimask_d, indp_d, out_d):
    nc = tc.nc

    with (
        tc.tile_pool(name="consts", bufs=1) as consts,
        tc.tile_pool(name="dram", bufs=1, space="DRAM") as dram,  # noqa: F841
    ):
        # ---- constants resident for the whole kernel ----
        # (sixth-0 x inputs are hoisted in front of the big wout const so
        # the first projection matmuls aren't starved behind it)
        xT8_r = xT8_d.rearrange("(kc p) s -> p kc s", p=128)
        xT_r = xT_d.rearrange("(kc p) s -> p kc s", p=128)
        bqk_sb = consts.tile([128, 16], F32)
        nc.gpsimd.dma_start(bqk_sb[:], bqk_d[:])
        bvb_sb = consts.tile([128, D], FP16)
        nc.gpsimd.dma_start(bvb_sb[:], bvb_d.to_broadcast((128, D)))
        boutb_sb = consts.tile([128, D], FP16)
        nc.gpsimd.dma_start(boutb_sb[:], boutb_d.to_broadcast((128, D)))
        # band mask for halo chunks 0/2 only (center chunk is always valid),
        # duplicated over the head-pair dim so one op masks both heads
        trimask_sb = consts.tile([128, 2, 2, W], FP16)
        for h2 in range(2):
            nc.gpsimd.dma_start(
                trimask_sb[:, h2],
                trimask_d.rearrange("p (c w) -> p c w", c=2),
            )
        ind_sb = consts.tile([128, T_EXT], F32)
        nc.gpsimd.dma_start(ind_sb[:], indp_d.rearrange("t p -> p t"))
        ident_sb = consts.tile([128, 128], FP16)
        make_identity(nc, ident_sb[:])
        wout_sb = consts.tile([128, KC, D], FP16)

        # ---- persistent SBUF stores for q/k/v (no DRAM spill) ----
        with (
            tc.tile_pool(name="stores", bufs=1) as stores,
            tc.tile_pool(name="p2_e", bufs=4) as p2_e,
            tc.tile_pool(name="p2_sm", bufs=6) as p2_sm,
            tc.tile_pool(name="p2_ao", bufs=2) as p2_ao,
            tc.tile_pool(name="p2_out", bufs=2) as p2_out,
            tc.tile_pool(name="p1_ps", bufs=2, space="PSUM") as p1_ps,
            tc.tile_pool(name="ps_s", bufs=2, space="PSUM") as ps_s_pool,
            tc.tile_pool(name="ps_ot", bufs=2, space="PSUM") as ps_ot_pool,
            tc.tile_pool(name="p1_w", bufs=1) as p1_w,
            tc.tile_pool(name="p1_x", bufs=2) as p1_x,
        ):
            # q/k in (feature-pair partitions x positions); k in fp8 so the
            # score LDWEIGHTS runs at 4x FWL rate (the row-tiled score pairs
            # are LDW-bound otherwise); v in (positions x head x 65) with
            # the indicator column
            q_store = stores.tile([128, KC, S_EXT], FP16)
            k_store = stores.tile([128, KC, S_EXT], FP8)
            v_store = stores.tile([128, T_EXT, H, VCOL], FP16)

            # x8 is fully SBUF-resident, loaded per-kc so each DMA moves
            # contiguous 2304B rows, split across both HW DMA queues so the
            # first q/k m-block (which needs all 8 kc chunks) isn't gated on
            # one queue. (gpsimd DMAs ride the slow software DGE — avoid.)
            # kc 0/1 ride the fast scalar HW queue so the first matmul can
            # start early; the rest go through the gpsimd software DGE,
            # keeping both HW queues free for xT / weights / output
            x8_sb = stores.tile([128, KC, S_EXT], FP8)
            for kc in range(2):
                nc.scalar.dma_start(
                    x8_sb[:, kc, :], xT8_d[kc * 128 : (kc + 1) * 128, :]
                )
            for kc in range(2, KC):
                nc.gpsimd.dma_start(
                    x8_sb[:, kc, :], xT8_d[kc * 128 : (kc + 1) * 128, :]
                )

            def load_x(sixth):
                tp0 = sixth * SIXTH
                xT_sb = p1_x.tile([128, KC, SIXTH], FP16, tag="xt", name="xT_sb")
                for h in range(2):
                    nc.scalar.dma_start(
                        xT_sb[:, 4 * h : 4 * h + 4, :],
                        xT_r[:, 4 * h : 4 * h + 4, tp0 : tp0 + SIXTH],
                    )
                return xT_sb

            x_hoisted = load_x(0)

            # weights: fp8 q/k + fp16 v weights on sync as per-(piece,kc)
            # contiguous-block DMAs (parallel across DMA engines); the first
            # matmuls only wait for w8 piece 0. wout (needed last) on gpsimd
            # after the sixth-0 x loads.
            w8_sb = p1_w.tile([128, KC, NQK], FP8)
            wv_sb = p1_w.tile([128, KC, D], FP16)

            def load_w8_piece(piece):
                for kc in range(KC):
                    r0 = (piece * KC + kc) * 128
                    nc.sync.dma_start(
                        w8_sb[:, kc, piece * 512 : (piece + 1) * 512],
                        wqk8_d[r0 : r0 + 128, :],
                    )

            def load_wv_piece(piece):
                for kc in range(KC):
                    r0 = (piece * KC + kc) * 128
                    nc.sync.dma_start(
                        wv_sb[:, kc, piece * 512 : (piece + 1) * 512],
                        wv_d[r0 : r0 + 128, :],
                    )

            load_w8_piece(0)
            load_w8_piece(1)
            load_w8_piece(2)
            load_w8_piece(3)
            load_wv_piece(0)
            load_wv_piece(1)
            for kc in range(KC):
                nc.gpsimd.dma_start(
                    wout_sb[:, kc, :], wout_d[kc * 128 : (kc + 1) * 128, :]
                )

            # ---- attention emission (software-pipelined per sixth) ----
            def emit_scores(e, a):
                """QK^T for both heads of pair a, + exp + band mask.
                Returns the masked exp tile."""
                # scoresT for both heads of the pair; the 4th c-slot is
                # padding so each head owns exactly one PSUM bank (the
                # two heads' matmuls run concurrently via row tiling)
                ps_s = ps_s_pool.tile([128, 2, 4, W], F32, tag="ps_s", name="ps_s")
                for c in range(3):
                    t = e - 1 + c
                    for h2 in range(2):
                        pr = slice(64 * h2, 64 * h2 + 64)
                        nc.tensor.matmul(
                            ps_s[:, h2, c, :],
                            lhsT=k_store[pr, a, t * 128 : (t + 1) * 128],
                            rhs=q_store[pr, a, e * 128 : (e + 1) * 128],
                            start=True,
                            stop=True,
                        )
                e_sb = p2_e.tile([128, 2, 3, W], FP16, name="e_sb")
                nc.scalar.activation(
                    e_sb[:],
                    ps_s[:, :, 0:3, :],
                    mybir.ActivationFunctionType.Exp,
                )
                # zero the out-of-band triangles of the halo chunks (0/2)
                nc.vector.tensor_mul(
                    e_sb[:, :, 0:3:2, :], e_sb[:, :, 0:3:2, :], trimask_sb[:]
                )
                return e_sb

            def emit_pv(e, a, e_sb):
                """e^T @ v for both heads (with indicator/denominator in
                column 64), then normalize. Returns the normalized tile."""
                ps_o = ps_ot_pool.tile([128, 512], F32, tag="ot", name="ps_o")
                for h2 in range(2):
                    for c in range(3):
                        nc.tensor.matmul(
                            ps_o[:, 65 * h2 : 65 * h2 + VCOL],
                            lhsT=e_sb[:, h2, c, :],
                            rhs=v_store[:, e - 1 + c, 2 * a + h2, :],
                            start=(c == 0),
                            stop=(c == 2),
                        )
                pv = ps_o[:, 0:130].rearrange("p (h v) -> p h v", h=2)
                rcp = p2_sm.tile([128, 2], F32, tag="rcp", name="rcp")
                nc.vector.reciprocal(rcp[:], pv[:, :, DH])
                no_sb = p2_sm.tile([128, 2, DH], FP16, tag="no", name="no_sb")
                nc.vector.tensor_mul(
                    no_sb[:],
                    pv[:, :, 0:DH],
                    rcp[:, :, None].broadcast_to((128, 2, DH)),
                )
                return no_sb

            def emit_transpose(a, no_sb, aoT_sb):
                """aoT = no^T via a normal matmul against the identity:
                full-rate, no transpose-mode switch. Emitted AFTER the next
                prefetched score block so its wait on the DVE norm overlaps
                independent PE work."""
                ps_t = ps_ot_pool.tile([128, 512], F32, tag="ot", name="ps_t")
                nc.tensor.matmul(
                    ps_t[:, :128],
                    lhsT=no_sb.rearrange("p a d -> p (a d)"),
                    rhs=ident_sb[:],
                    start=True,
                    stop=True,
                )
                if a % 2 == 0:
                    nc.scalar.copy(aoT_sb[:, a, :], ps_t[:, :128])
                else:
                    nc.vector.tensor_copy(aoT_sb[:, a, :], ps_t[:, :128])

            def emit_outproj(e, aoT_sb):
                for g in range(2):
                    ps_f = ps_ot_pool.tile([128, 512], F32, tag="ot", name="ps_f")
                    for a in range(HP):
                        nc.tensor.matmul(
                            ps_f[:],
                            lhsT=aoT_sb[:, a, :],
                            rhs=wout_sb[:, a, g * 512 : (g + 1) * 512],
                            start=(a == 0),
                            stop=(a == HP - 1),
                        )
                    fo = p2_out.tile([128, 512], F32, tag="fo", name="fo")
                    nc.vector.tensor_add(
                        fo[:], ps_f[:], boutb_sb[:, g * 512 : (g + 1) * 512]
                    )
                    nc.sync.dma_start(
                        out_d[(e - 1) * 128 : e * 128, g * 512 : (g + 1) * 512],
                        fo[:],
                    )

            def emit_attention(tiles):
                """Pipelined attention over this sixth's ready query tiles:
                scores/exp/mask run 2 head-pairs ahead of PV so the PE never
                waits on the ACT/DVE softmax chain."""
                work = [(e, a) for e in tiles for a in range(HP)]
                pending = {}
                aoT = {}
                for e in tiles:
                    aoT[e] = p2_ao.tile([128, HP, 128], FP16, name="aoT")
                for i in range(min(2, len(work))):
                    pending[work[i]] = emit_scores(*work[i])
                for i, (e, a) in enumerate(work):
                    e_sb = pending.pop((e, a))
                    no_sb = emit_pv(e, a, e_sb)
                    if i + 2 < len(work):
                        pending[work[i + 2]] = emit_scores(*work[i + 2])
                    emit_transpose(a, no_sb, aoT[e])
                    if a == HP - 1:
                        emit_outproj(e, aoT.pop(e))

            def emit_v(sixth, tt, xT_sb):
                """v projection for tile t: fp16, out layout (positions x
                features), indicator in column 64."""
                t = sixth * (SIXTH // 128) + tt
                for g in range(2):
                    ps = p1_ps.tile([128, 512], F32, tag="ps1", name="ps")
                    for kc in range(KC):
                        nc.tensor.matmul(
                            ps[:],
                            lhsT=xT_sb[:, kc, tt * 128 : (tt + 1) * 128],
                            rhs=wv_sb[:, kc, g * 512 : (g + 1) * 512],
                            start=(kc == 0),
                            stop=(kc == KC - 1),
                        )
                    nc.vector.tensor_add(
                        v_store[:, t, g * 8 : (g + 1) * 8, 0:DH],
                        ps.rearrange("p (h d) -> p h d", d=DH),
                        bvb_sb[:, g * 512 : (g + 1) * 512].rearrange(
                            "p (h d) -> p h d", d=DH
                        ),
                    )
                nc.vector.memset(v_store[:, t, :, DH : DH + 1], 1.0)
                # zero v and indicator at out-of-sequence positions
                nc.vector.tensor_scalar_mul(
                    v_store[:, t], v_store[:, t], ind_sb[:, t : t + 1]
                )

            def emit_qk(sixth):
                """q/k: fp8 DoubleRow matmuls, out layout (feature-chunk
                partitions x positions), written straight into the
                persistent stores. q is not needed for the halo tiles
                (first/last 128 positions)."""
                tp0 = sixth * SIXTH
                for m in range(16):
                    dst = q_store if m < KC else k_store
                    mm = m if m < KC else m - KC
                    o0, on = 0, SIXTH
                    if m < KC and sixth == 0:
                        o0, on = 128, SIXTH - 128
                    elif m < KC and sixth == 5:
                        o0, on = 0, SIXTH - 128
                    ps = p1_ps.tile([128, 512], F32, tag="ps1", name="ps")
                    for k2 in range(KC2):
                        nc.tensor.matmul(
                            ps[:, :on],
                            lhsT=w8_sb[:, 2 * k2 : 2 * k2 + 2, m * 128 : (m + 1) * 128],
                            rhs=x8_sb[:, 2 * k2 : 2 * k2 + 2, tp0 + o0 : tp0 + o0 + on],
                            start=(k2 == 0),
                            stop=(k2 == KC2 - 1),
                            perf_mode=mybir.MatmulPerfMode.DoubleRow,
                        )
                    nc.scalar.activation(
                        dst[:, mm, tp0 + o0 : tp0 + o0 + on],
                        ps[:, :on],
                        mybir.ActivationFunctionType.Identity,
                        bias=bqk_sb[:, m : m + 1],
                        scale=Q_SCALE if m < KC else QK_DESCALE,
                    )

            # The q/k projection runs one sixth AHEAD of the v/attention
            # stream: it depends only on the resident x8 + w8, so the two
            # front-loaded sixths give the DMA queues a ~27us dense-compute
            # window to land wv/xT/wout before v/attention needs them.
            # v tiles stay interleaved with ready attention tiles so the PE
            # has projection work while the softmax chain (ACT/DVE) catches
            # up, and the last sixth's attention isn't all serialized at
            # the end.
            emit_qk(0)
            emit_qk(1)
            xT_cur = x_hoisted
            for sixth in range(6):
                xT_next = load_x(sixth + 1) if sixth < 5 else None
                if sixth == 0:
                    for tt in range(3):
                        emit_v(sixth, tt, xT_cur)
                    emit_attention([1])
                else:
                    for tt in range(3):
                        emit_v(sixth, tt, xT_cur)
                        emit_attention([3 * sixth - 1 + tt])
                if sixth + 2 <= 5:
                    emit_qk(sixth + 2)
                xT_cur = xT_next


_NC_CACHE = None


def _get_nc():
    global _NC_CACHE
    if _NC_CACHE is None:
        _NC_CACHE = _build_nc()
    return _NC_CACHE


def _host_inputs(x, Wqkv, bqkv, Wout, bout):
    """Build the 8 per-core input maps."""
    x = np.asarray(x, dtype=np.float32)
    Wqkv = np.asarray(Wqkv, dtype=np.float32)
    bqkv = np.asarray(bqkv, dtype=np.float32)
    Wout = np.asarray(Wout, dtype=np.float32)
    bout = np.asarray(bout, dtype=np.float32)

    scale = np.float32(1.0 / np.sqrt(D))
    bs = bqkv.copy()
    bs[:D] *= scale

    bqk = np.ascontiguousarray(bs[:NQK].reshape(16, 128).T)  # (128, 16)
    bvb = bs[NQK:].reshape(1, D)
    boutb = bout.reshape(1, D)

    # pre-shuffle weights into (piece, kc, 128, 512) contiguous DMA blocks
    wqk8 = np.clip(Wqkv[:, :NQK] * W_SCALE, -240.0, 240.0).astype(
        ml_dtypes.float8_e4m3
    )
    wqk8 = np.ascontiguousarray(
        wqk8.reshape(KC, 128, 4, 512).transpose(2, 0, 1, 3)
    ).reshape(4 * KC * 128, 512)
    wv = Wqkv[:, NQK:].astype(np.float16)
    wv = np.ascontiguousarray(
        wv.reshape(KC, 128, 2, 512).transpose(2, 0, 1, 3)
    ).reshape(2 * KC * 128, 512)

    # band mask for halo chunks 0/2 in (j_within_chunk, chunk, i) layout
    jc = np.arange(128)[:, None]
    i = np.arange(128)[None, :]
    tm = np.ones((128, 2, 128), dtype=np.float32)
    tm[:, 0] = (jc >= i).astype(np.float32)
    tm[:, 1] = (jc <= i).astype(np.float32)
    trimask = tm.reshape(128, 2 * W).astype(np.float16)

    in_maps = []
    for core in range(N_CORES):
        b, half = core // 2, core % 2
        s0 = half * S_LOC
        lo, hi = s0 - W, s0 + S_LOC + W
        xp = np.zeros((S_EXT, D), dtype=np.float32)
        src_lo, src_hi = max(lo, 0), min(hi, S)
        xp[src_lo - lo : src_hi - lo] = x[b, src_lo:src_hi]
        xT = np.ascontiguousarray(xp.T)
        xT8 = np.clip(xT * X_SCALE, -240.0, 240.0).astype(ml_dtypes.float8_e4m3)

        valid = np.ones(S_EXT, dtype=np.float32)
        if lo < 0:
            valid[: -lo] = 0.0
        if hi > S:
            valid[S - hi :] = 0.0
        indp = np.ascontiguousarray(valid.reshape(T_EXT, 128))

        in_maps.append(
            {
                "xT": xT.astype(np.float16),
                "xT8": xT8,
                "wqk8": wqk8,
                "wv": wv,
                "bqk": bqk,
                "bvb": bvb.astype(np.float16),
                "wout": Wout.astype(np.float16),
                "boutb": boutb.astype(np.float16),
                "trimask": trimask,
                "indp": indp,
            }
        )
    return in_maps


def kernel(x, Wqkv, bqkv, Wout, bout, _trace=False, _trace_cores=None):
    in_maps = _host_inputs(x, Wqkv, bqkv, Wout, bout)
    nc = _get_nc()
    res = run_bass_kernel_spmd(
        nc,
        in_maps,
        list(range(N_CORES)),
        trace=_trace,
        trace_cores=_trace_cores,
    )
    out = np.empty((B, S, D), dtype=np.float32)
    for core in range(N_CORES):
        b, half = core // 2, core % 2
        s0 = half * S_LOC
        out[b, s0 : s0 + S_LOC] = res.results[core]["out"]
    if _trace:
        return out, res
    return out


# revision 29
# speedup vs baseline: 1.0129x; 1.0129x over previous
"""LocalWindowAttention TRN2 kernel.

Full inputs -> full output. Sharding: 8 cores = batch(4) x seq-half(2).
Each core computes 2048 query positions; k/v halos (128 each side) come
from overlapping the per-core x slice, so no collectives are needed.

Math (per core, matching reference):
  qkv = x @ Wqkv + bqkv  (q pre-scaled by 1/sqrt(1024) via the store scale)
  banded attention, window 128, block size 128: query tile e attends key
  tiles e-1, e, e+1 with a static band mask |kpos - qpos| <= 128.
  Softmax without max-subtraction (scores are O(0.1)); invalid keys are
  zeroed AFTER exp via a 0/1 band mask (only the halo chunks 0/2 need
  masking; the center chunk is always fully in-band), out-of-sequence
  keys are zeroed via a validity indicator carried as a 65th column of v
  (which also yields the softmax denominator through the PV matmul).
  out = attn_out @ Wout + bout

Dtypes: q/k projection runs in fp8e4m3 DoubleRow (2x PE rate); inputs are
pre-scaled on the host (x*16, Wqk*64) to avoid the e4m3 subnormal range and
the scale is divided back out in the PSUM->SBUF store. The fp8 error only
perturbs attention scores (softmax attenuates it); v / PV / out-proj stay
fp16 since their quantization error would hit the output 1:1.
"""

import sys

import numpy as np

for _p in ("/opt/trn_rl_repo",):
    if _p not in sys.path:
        sys.path.insert(0, _p)

import ml_dtypes  # noqa: E402

import concourse.bass as bass  # noqa: E402,F401
import concourse.mybir as mybir  # noqa: E402
import concourse.tile as tile  # noqa: E402
from concourse import bacc  # noqa: E402
from concourse.bass_utils import run_bass_kernel_spmd  # noqa: E402
from concourse.masks import make_identity  # noqa: E402

F32 = mybir.dt.float32
FP16 = mybir.dt.float16
FP8 = mybir.dt.float8e4

B, S, D = 4, 4096, 1024
H, DH, W = 16, 64, 128
N_CORES = 8
S_LOC = 2048            # query positions per core
T_Q = S_LOC // W        # 16 query tiles per core
T_EXT = T_Q + 2         # 18 extended tiles (with halo)
S_EXT = T_EXT * W       # 2304
NQK = 2 * D             # q+k projected features
KC = D // 128           # 8 contraction chunks
KC2 = KC // 2           # 4 double-row chunks
HP = H // 2             # 8 head pairs
VCOL = DH + 1           # 64 v dims + indicator column

SIXTH = S_EXT // 6      # 384 positions per xT streaming chunk

X_SCALE = 16.0          # host pre-scale on x for fp8 (avoid subnormals)
W_SCALE = 64.0          # host pre-scale on Wqk for fp8
QK_DESCALE = 1.0 / (X_SCALE * W_SCALE)
Q_SCALE = QK_DESCALE / np.sqrt(D)   # extra 1/sqrt(d_model) on q


def _build_nc():
    nc = bacc.Bacc(
        "TRN2",
        target_bir_lowering=False,
        debug=False,
        num_devices=N_CORES,
    )

    xT_d = nc.dram_tensor("xT", [D, S_EXT], FP16, kind="ExternalInput").ap()
    xT8_d = nc.dram_tensor("xT8", [D, S_EXT], FP8, kind="ExternalInput").ap()
    # weights arrive pre-shuffled as (piece, kc, 128, 512) so every DMA
    # below reads one fully contiguous 64KB/128KB block
    wqk8_d = nc.dram_tensor("wqk8", [4 * KC * 128, 512], FP8, kind="ExternalInput").ap()
    wv_d = nc.dram_tensor("wv", [2 * KC * 128, 512], FP16, kind="ExternalInput").ap()
    bqk_d = nc.dram_tensor("bqk", [128, 16], F32, kind="ExternalInput").ap()
    bvb_d = nc.dram_tensor("bvb", [1, D], FP16, kind="ExternalInput").ap()
    wout_d = nc.dram_tensor("wout", [D, D], FP16, kind="ExternalInput").ap()
    boutb_d = nc.dram_tensor("boutb", [1, D], FP16, kind="ExternalInput").ap()
    trimask_d = nc.dram_tensor("trimask", [128, 2 * W], FP16, kind="ExternalInput").ap()
    indp_d = nc.dram_tensor("indp", [T_EXT, 128], F32, kind="ExternalInput").ap()
    # fp16 output halves the 8MB/core output DMA (negligible extra error;
    # the host converts back to fp32)
    out_d = nc.dram_tensor("out", [S_LOC, D], FP16, kind="ExternalOutput").ap()

    with tile.TileContext(nc) as tc:
        _emit(tc, xT_d, xT8_d, wqk8_d, wv_d, bqk_d, bvb_d, wout_d, boutb_d,
              trimask_d, indp_d, out_d)
    nc.compile()
    return nc


def _emit(tc, xT_d, xT8_d, wqk8_d, wv_d, bqk_d, bvb_d, wout_d, boutb_d,
          trimask_d, indp_d, out_d):
    nc = tc.nc

    with (
        tc.tile_pool(name="consts", bufs=1) as consts,
        tc.tile_pool(name="dram", bufs=1, space="DRAM") as dram,  # noqa: F841
    ):
        # ---- constants resident for the whole kernel ----
        # (sixth-0 x inputs are hoisted in front of the big wout const so
        # the first projection matmuls aren't starved behind it)
        xT8_r = xT8_d.rearrange("(kc p) s -> p kc s", p=128)
        xT_r = xT_d.rearrange("(kc p) s -> p kc s", p=128)
        bqk_sb = consts.tile([128, 16], F32)
        nc.gpsimd.dma_start(bqk_sb[:], bqk_d[:])
        bvb_sb = consts.tile([128, D], FP16)
        nc.gpsimd.dma_start(bvb_sb[:], bvb_d.to_broadcast((128, D)))
        boutb_sb = consts.tile([128, D], FP16)
        nc.gpsimd.dma_start(boutb_sb[:], boutb_d.to_broadcast((128, D)))
        # band mask for halo chunks 0/2 only (center chunk is always valid),
        # duplicated over the head-pair dim so one op masks both heads
        trimask_sb = consts.tile([128, 2, 2, W], FP16)
        for h2 in range(2):
            nc.gpsimd.dma_start(
                trimask_sb[:, h2],
                trimask_d.rearrange("p (c w) -> p c w", c=2),
            )
        ind_sb = consts.tile([128, T_EXT], F32)
        nc.gpsimd.dma_start(ind_sb[:], indp_d.rearrange("t p -> p t"))
        ident_sb = consts.tile([128, 128], FP16)
        make_identity(nc, ident_sb[:])
        wout_sb = consts.tile([128, KC, D], FP16)

        # ---- persistent SBUF stores for q/k/v (no DRAM spill) ----
        with (
            tc.tile_pool(name="stores", bufs=1) as stores,
            tc.tile_pool(name="p2_e", bufs=4) as p2_e,
            tc.tile_pool(name="p2_sm", bufs=6) as p2_sm,
            tc.tile_pool(name="p2_ao", bufs=2) as p2_ao,
            tc.tile_pool(name="p2_out", bufs=2) as p2_out,
            tc.tile_pool(name="p1_ps", bufs=2, space="PSUM") as p1_ps,
            tc.tile_pool(name="ps_s", bufs=2, space="PSUM") as ps_s_pool,
            tc.tile_pool(name="ps_ot", bufs=2, space="PSUM") as ps_ot_pool,
            tc.tile_pool(name="p1_w", bufs=1) as p1_w,
            tc.tile_pool(name="p1_x", bufs=2) as p1_x,
        ):
            # q/k in (feature-pair partitions x positions); k in fp8 so the
            # score LDWEIGHTS runs at 4x FWL rate (the row-tiled score pairs
            # are LDW-bound otherwise); v in (positions x head x 65) with
            # the indicator column
            q_store = stores.tile([128, KC, S_EXT], FP16)
            k_store = stores.tile([128, KC, S_EXT], FP8)
            v_store = stores.tile([128, T_EXT, H, VCOL], FP16)

            # x8 is fully SBUF-resident, loaded per-kc so each DMA moves
            # contiguous 2304B rows, split across both HW DMA queues so the
            # first q/k m-block (which needs all 8 kc chunks) isn't gated on
            # one queue. (gpsimd DMAs ride the slow software DGE — avoid.)
            x8_sb = stores.tile([128, KC, S_EXT], FP8)
            for kc in range(4):
                nc.scalar.dma_start(
                    x8_sb[:, kc, :], xT8_d[kc * 128 : (kc + 1) * 128, :]
                )

            def load_x(sixth):
                tp0 = sixth * SIXTH
                xT_sb = p1_x.tile([128, KC, SIXTH], FP16, tag="xt", name="xT_sb")
                for h in range(2):
                    nc.scalar.dma_start(
                        xT_sb[:, 4 * h : 4 * h + 4, :],
                        xT_r[:, 4 * h : 4 * h + 4, tp0 : tp0 + SIXTH],
                    )
                return xT_sb

            x_hoisted = load_x(0)

            # weights: fp8 q/k + fp16 v weights on sync as per-(piece,kc)
            # contiguous-block DMAs (parallel across DMA engines); the first
            # matmuls only wait for w8 piece 0. wout (needed last) on gpsimd
            # after the sixth-0 x loads.
            w8_sb = p1_w.tile([128, KC, NQK], FP8)
            wv_sb = p1_w.tile([128, KC, D], FP16)

            def load_w8_piece(piece):
                for kc in range(KC):
                    r0 = (piece * KC + kc) * 128
                    nc.sync.dma_start(
                        w8_sb[:, kc, piece * 512 : (piece + 1) * 512],
                        wqk8_d[r0 : r0 + 128, :],
                    )

            def load_wv_piece(piece):
                for kc in range(KC):
                    r0 = (piece * KC + kc) * 128
                    nc.sync.dma_start(
                        wv_sb[:, kc, piece * 512 : (piece + 1) * 512],
                        wv_d[r0 : r0 + 128, :],
                    )

            load_w8_piece(0)
            for kc in range(4, KC):
                nc.sync.dma_start(
                    x8_sb[:, kc, :], xT8_d[kc * 128 : (kc + 1) * 128, :]
                )
            load_w8_piece(1)
            load_w8_piece(2)
            load_w8_piece(3)
            load_wv_piece(0)
            load_wv_piece(1)
            for kc in range(KC):
                nc.sync.dma_start(
                    wout_sb[:, kc, :], wout_d[kc * 128 : (kc + 1) * 128, :]
                )

            # ---- attention emission (software-pipelined per sixth) ----
            def emit_scores(e, a):
                """QK^T for both heads of pair a, + exp + band mask.
                Returns the masked exp tile."""
                # scoresT for both heads of the pair; the 4th c-slot is
                # padding so each head owns exactly one PSUM bank (the
                # two heads' matmuls run concurrently via row tiling)
                ps_s = ps_s_pool.tile([128, 2, 4, W], F32, tag="ps_s", name="ps_s")
                for c in range(3):
                    t = e - 1 + c
                    for h2 in range(2):
                        pr = slice(64 * h2, 64 * h2 + 64)
                        nc.tensor.matmul(
                            ps_s[:, h2, c, :],
                            lhsT=k_store[pr, a, t * 128 : (t + 1) * 128],
                            rhs=q_store[pr, a, e * 128 : (e + 1) * 128],
                            start=True,
                            stop=True,
                        )
                e_sb = p2_e.tile([128, 2, 3, W], FP16, name="e_sb")
                nc.scalar.activation(
                    e_sb[:],
                    ps_s[:, :, 0:3, :],
                    mybir.ActivationFunctionType.Exp,
                )
                # zero the out-of-band triangles of the halo chunks (0/2)
                nc.vector.tensor_mul(
                    e_sb[:, :, 0:3:2, :], e_sb[:, :, 0:3:2, :], trimask_sb[:]
                )
                return e_sb

            def emit_pv(e, a, e_sb):
                """e^T @ v for both heads (with indicator/denominator in
                column 64), then normalize. Returns the normalized tile."""
                ps_o = ps_ot_pool.tile([128, 512], F32, tag="ot", name="ps_o")
                for h2 in range(2):
                    for c in range(3):
                        nc.tensor.matmul(
                            ps_o[:, 65 * h2 : 65 * h2 + VCOL],
                            lhsT=e_sb[:, h2, c, :],
                            rhs=v_store[:, e - 1 + c, 2 * a + h2, :],
                            start=(c == 0),
                            stop=(c == 2),
                        )
                pv = ps_o[:, 0:130].rearrange("p (h v) -> p h v", h=2)
                rcp = p2_sm.tile([128, 2], F32, tag="rcp", name="rcp")
                nc.vector.reciprocal(rcp[:], pv[:, :, DH])
                no_sb = p2_sm.tile([128, 2, DH], FP16, tag="no", name="no_sb")
                nc.vector.tensor_mul(
                    no_sb[:],
                    pv[:, :, 0:DH],
                    rcp[:, :, None].broadcast_to((128, 2, DH)),
                )
                return no_sb

            def emit_transpose(a, no_sb, aoT_sb):
                """aoT = no^T via a normal matmul against the identity:
                full-rate, no transpose-mode switch. Emitted AFTER the next
                prefetched score block so its wait on the DVE norm overlaps
                independent PE work."""
                ps_t = ps_ot_pool.tile([128, 512], F32, tag="ot", name="ps_t")
                nc.tensor.matmul(
                    ps_t[:, :128],
                    lhsT=no_sb.rearrange("p a d -> p (a d)"),
                    rhs=ident_sb[:],
                    start=True,
                    stop=True,
                )
                if a % 2 == 0:
                    nc.scalar.copy(aoT_sb[:, a, :], ps_t[:, :128])
                else:
                    nc.vector.tensor_copy(aoT_sb[:, a, :], ps_t[:, :128])

            def emit_outproj(e, aoT_sb):
                for g in range(2):
                    ps_f = ps_ot_pool.tile([128, 512], F32, tag="ot", name="ps_f")
                    for a in range(HP):
                        nc.tensor.matmul(
                            ps_f[:],
                            lhsT=aoT_sb[:, a, :],
                            rhs=wout_sb[:, a, g * 512 : (g + 1) * 512],
                            start=(a == 0),
                            stop=(a == HP - 1),
                        )
                    fo = p2_out.tile([128, 512], FP16, tag="fo", name="fo")
                    nc.vector.tensor_add(
                        fo[:], ps_f[:], boutb_sb[:, g * 512 : (g + 1) * 512]
                    )
                    nc.sync.dma_start(
                        out_d[(e - 1) * 128 : e * 128, g * 512 : (g + 1) * 512],
                        fo[:],
                    )

            def emit_attention(tiles):
                """Pipelined attention over this sixth's ready query tiles:
                scores/exp/mask run 2 head-pairs ahead of PV so the PE never
                waits on the ACT/DVE softmax chain."""
                work = [(e, a) for e in tiles for a in range(HP)]
                pending = {}
                aoT = {}
                for e in tiles:
                    aoT[e] = p2_ao.tile([128, HP, 128], FP16, name="aoT")
                for i in range(min(2, len(work))):
                    pending[work[i]] = emit_scores(*work[i])
                for i, (e, a) in enumerate(work):
                    e_sb = pending.pop((e, a))
                    no_sb = emit_pv(e, a, e_sb)
                    if i + 2 < len(work):
                        pending[work[i + 2]] = emit_scores(*work[i + 2])
                    emit_transpose(a, no_sb, aoT[e])
                    if a == HP - 1:
                        emit_outproj(e, aoT.pop(e))

            def emit_v(sixth, tt, xT_sb):
                """v projection for tile t: fp16, out layout (positions x
                features), indicator in column 64."""
                t = sixth * (SIXTH // 128) + tt
                for g in range(2):
                    ps = p1_ps.tile([128, 512], F32, tag="ps1", name="ps")
                    for kc in range(KC):
                        nc.tensor.matmul(
                            ps[:],
                            lhsT=xT_sb[:, kc, tt * 128 : (tt + 1) * 128],
                            rhs=wv_sb[:, kc, g * 512 : (g + 1) * 512],
                            start=(kc == 0),
                            stop=(kc == KC - 1),
                        )
                    nc.vector.tensor_add(
                        v_store[:, t, g * 8 : (g + 1) * 8, 0:DH],
                        ps.rearrange("p (h d) -> p h d", d=DH),
                        bvb_sb[:, g * 512 : (g + 1) * 512].rearrange(
                            "p (h d) -> p h d", d=DH
                        ),
                    )
                nc.vector.memset(v_store[:, t, :, DH : DH + 1], 1.0)
                # zero v and indicator at out-of-sequence positions
                nc.vector.tensor_scalar_mul(
                    v_store[:, t], v_store[:, t], ind_sb[:, t : t + 1]
                )

            def emit_qk(sixth):
                """q/k: fp8 DoubleRow matmuls, out layout (feature-chunk
                partitions x positions), written straight into the
                persistent stores. q is not needed for the halo tiles
                (first/last 128 positions)."""
                tp0 = sixth * SIXTH
                for m in range(16):
                    dst = q_store if m < KC else k_store
                    mm = m if m < KC else m - KC
                    o0, on = 0, SIXTH
                    if m < KC and sixth == 0:
                        o0, on = 128, SIXTH - 128
                    elif m < KC and sixth == 5:
                        o0, on = 0, SIXTH - 128
                    ps = p1_ps.tile([128, 512], F32, tag="ps1", name="ps")
                    for k2 in range(KC2):
                        nc.tensor.matmul(
                            ps[:, :on],
                            lhsT=w8_sb[:, 2 * k2 : 2 * k2 + 2, m * 128 : (m + 1) * 128],
                            rhs=x8_sb[:, 2 * k2 : 2 * k2 + 2, tp0 + o0 : tp0 + o0 + on],
                            start=(k2 == 0),
                            stop=(k2 == KC2 - 1),
                            perf_mode=mybir.MatmulPerfMode.DoubleRow,
                        )
                    nc.scalar.activation(
                        dst[:, mm, tp0 + o0 : tp0 + o0 + on],
                        ps[:, :on],
                        mybir.ActivationFunctionType.Identity,
                        bias=bqk_sb[:, m : m + 1],
                        scale=Q_SCALE if m < KC else QK_DESCALE,
                    )

            # The q/k projection runs one sixth AHEAD of the v/attention
            # stream: it depends only on the resident x8 + w8, so the two
            # front-loaded sixths give the DMA queues a ~27us dense-compute
            # window to land wv/xT/wout before v/attention needs them.
            # v tiles stay interleaved with ready attention tiles so the PE
            # has projection work while the softmax chain (ACT/DVE) catches
            # up, and the last sixth's attention isn't all serialized at
            # the end.
            emit_qk(0)
            emit_qk(1)
            xT_cur = x_hoisted
            for sixth in range(6):
                xT_next = load_x(sixth + 1) if sixth < 5 else None
                if sixth == 0:
                    for tt in range(3):
                        emit_v(sixth, tt, xT_cur)
                    emit_attention([1])
                else:
                    for tt in range(3):
                        emit_v(sixth, tt, xT_cur)
                        emit_attention([3 * sixth - 1 + tt])
                if sixth + 2 <= 5:
                    emit_qk(sixth + 2)
                xT_cur = xT_next


_NC_CACHE = None


def _get_nc():
    global _NC_CACHE
    if _NC_CACHE is None:
        _NC_CACHE = _build_nc()
    return _NC_CACHE


def _host_inputs(x, Wqkv, bqkv, Wout, bout):
    """Build the 8 per-core input maps."""
    x = np.asarray(x, dtype=np.float32)
    Wqkv = np.asarray(Wqkv, dtype=np.float32)
    bqkv = np.asarray(bqkv, dtype=np.float32)
    Wout = np.asarray(Wout, dtype=np.float32)
    bout = np.asarray(bout, dtype=np.float32)

    scale = np.float32(1.0 / np.sqrt(D))
    bs = bqkv.copy()
    bs[:D] *= scale

    bqk = np.ascontiguousarray(bs[:NQK].reshape(16, 128).T)  # (128, 16)
    bvb = bs[NQK:].reshape(1, D)
    boutb = bout.reshape(1, D)

    # pre-shuffle weights into (piece, kc, 128, 512) contiguous DMA blocks
    wqk8 = np.clip(Wqkv[:, :NQK] * W_SCALE, -240.0, 240.0).astype(
        ml_dtypes.float8_e4m3
    )
    wqk8 = np.ascontiguousarray(
        wqk8.reshape(KC, 128, 4, 512).transpose(2, 0, 1, 3)
    ).reshape(4 * KC * 128, 512)
    wv = Wqkv[:, NQK:].astype(np.float16)
    wv = np.ascontiguousarray(
        wv.reshape(KC, 128, 2, 512).transpose(2, 0, 1, 3)
    ).reshape(2 * KC * 128, 512)

    # band mask for halo chunks 0/2 in (j_within_chunk, chunk, i) layout
    jc = np.arange(128)[:, None]
    i = np.arange(128)[None, :]
    tm = np.ones((128, 2, 128), dtype=np.float32)
    tm[:, 0] = (jc >= i).astype(np.float32)
    tm[:, 1] = (jc <= i).astype(np.float32)
    trimask = tm.reshape(128, 2 * W).astype(np.float16)

    in_maps = []
    for core in range(N_CORES):
        b, half = core // 2, core % 2
        s0 = half * S_LOC
        lo, hi = s0 - W, s0 + S_LOC + W
        xp = np.zeros((S_EXT, D), dtype=np.float32)
        src_lo, src_hi = max(lo, 0), min(hi, S)
        xp[src_lo - lo : src_hi - lo] = x[b, src_lo:src_hi]
        xT = np.ascontiguousarray(xp.T)
        xT8 = np.clip(xT * X_SCALE, -240.0, 240.0).astype(ml_dtypes.float8_e4m3)

        valid = np.ones(S_EXT, dtype=np.float32)
        if lo < 0:
            valid[: -lo] = 0.0
        if hi > S:
            valid[S - hi :] = 0.0
        indp = np.ascontiguousarray(valid.reshape(T_EXT, 128))

        in_maps.append(
            {
                "xT": xT.astype(np.float16),
                "xT8": xT8,
                "wqk8": wqk8,
                "wv": wv,
                "bqk": bqk,
                "bvb": bvb.astype(np.float16),
                "wout": Wout.astype(np.float16),
                "boutb": boutb.astype(np.float16),
                "trimask": trimask,
                "indp": indp,
            }
        )
    return in_maps


def kernel(x, Wqkv, bqkv, Wout, bout, _trace=False, _trace_cores=None):
    in_maps = _host_inputs(x, Wqkv, bqkv, Wout, bout)
    nc = _get_nc()
    res = run_bass_kernel_spmd(
        nc,
        in_maps,
        list(range(N_CORES)),
        trace=_trace,
        trace_cores=_trace_cores,
    )
    out = np.empty((B, S, D), dtype=np.float32)
    for core in range(N_CORES):
        b, half = core // 2, core % 2
        s0 = half * S_LOC
        out[b, s0 : s0 + S_LOC] = res.results[core]["out"]
    if _trace:
        return out, res
    return out


# revision 30
# speedup vs baseline: 1.0600x; 1.0465x over previous
"""LocalWindowAttention TRN2 kernel.

Full inputs -> full output. Sharding: 8 cores = batch(4) x seq-half(2).
Each core computes 2048 query positions; k/v halos (128 each side) come
from overlapping the per-core x slice, so no collectives are needed.

Math (per core, matching reference):
  qkv = x @ Wqkv + bqkv  (q pre-scaled by 1/sqrt(1024) via the store scale)
  banded attention, window 128, block size 128: query tile e attends key
  tiles e-1, e, e+1 with a static band mask |kpos - qpos| <= 128.
  Softmax without max-subtraction (scores are O(0.1)); invalid keys are
  zeroed AFTER exp via a 0/1 band mask (only the halo chunks 0/2 need
  masking; the center chunk is always fully in-band), out-of-sequence
  keys are zeroed via a validity indicator carried as a 65th column of v
  (which also yields the softmax denominator through the PV matmul).
  out = attn_out @ Wout + bout

Dtypes: q/k projection runs in fp8e4m3 DoubleRow (2x PE rate); inputs are
pre-scaled on the host (x*16, Wqk*64) to avoid the e4m3 subnormal range and
the scale is divided back out in the PSUM->SBUF store. The fp8 error only
perturbs attention scores (softmax attenuates it); v / PV / out-proj stay
fp16 since their quantization error would hit the output 1:1.
"""

import sys

import numpy as np

for _p in ("/opt/trn_rl_repo",):
    if _p not in sys.path:
        sys.path.insert(0, _p)

import ml_dtypes  # noqa: E402

import concourse.bass as bass  # noqa: E402,F401
import concourse.mybir as mybir  # noqa: E402
import concourse.tile as tile  # noqa: E402
from concourse import bacc  # noqa: E402
from concourse.bass_utils import run_bass_kernel_spmd  # noqa: E402
from concourse.masks import make_identity  # noqa: E402

F32 = mybir.dt.float32
FP16 = mybir.dt.float16
FP8 = mybir.dt.float8e4

B, S, D = 4, 4096, 1024
H, DH, W = 16, 64, 128
N_CORES = 8
S_LOC = 2048            # query positions per core
T_Q = S_LOC // W        # 16 query tiles per core
T_EXT = T_Q + 2         # 18 extended tiles (with halo)
S_EXT = T_EXT * W       # 2304
NQK = 2 * D             # q+k projected features
KC = D // 128           # 8 contraction chunks
KC2 = KC // 2           # 4 double-row chunks
HP = H // 2             # 8 head pairs
VCOL = DH + 1           # 64 v dims + indicator column

SIXTH = S_EXT // 6      # 384 positions per xT streaming chunk

X_SCALE = 16.0          # host pre-scale on x for fp8 (avoid subnormals)
W_SCALE = 64.0          # host pre-scale on Wqk for fp8
QK_DESCALE = 1.0 / (X_SCALE * W_SCALE)
Q_SCALE = QK_DESCALE / np.sqrt(D)   # extra 1/sqrt(d_model) on q


def _build_nc():
    nc = bacc.Bacc(
        "TRN2",
        target_bir_lowering=False,
        debug=False,
        num_devices=N_CORES,
    )

    xT_d = nc.dram_tensor("xT", [D, S_EXT], FP16, kind="ExternalInput").ap()
    xT8_d = nc.dram_tensor("xT8", [D, S_EXT], FP8, kind="ExternalInput").ap()
    # weights arrive pre-shuffled as (piece, kc, 128, 512) so every DMA
    # below reads one fully contiguous 64KB/128KB block
    wqk8_d = nc.dram_tensor("wqk8", [4 * KC * 128, 512], FP8, kind="ExternalInput").ap()
    wv_d = nc.dram_tensor("wv", [2 * KC * 128, 512], FP16, kind="ExternalInput").ap()
    bqk_d = nc.dram_tensor("bqk", [128, 16], F32, kind="ExternalInput").ap()
    bvb_d = nc.dram_tensor("bvb", [1, D], FP16, kind="ExternalInput").ap()
    wout_d = nc.dram_tensor("wout", [D, D], FP16, kind="ExternalInput").ap()
    boutb_d = nc.dram_tensor("boutb", [1, D], FP16, kind="ExternalInput").ap()
    trimask_d = nc.dram_tensor("trimask", [128, 2 * W], FP16, kind="ExternalInput").ap()
    indp_d = nc.dram_tensor("indp", [T_EXT, 128], F32, kind="ExternalInput").ap()
    # fp16 output halves the 8MB/core output DMA (negligible extra error;
    # the host converts back to fp32)
    out_d = nc.dram_tensor("out", [S_LOC, D], FP16, kind="ExternalOutput").ap()

    with tile.TileContext(nc) as tc:
        _emit(tc, xT_d, xT8_d, wqk8_d, wv_d, bqk_d, bvb_d, wout_d, boutb_d,
              trimask_d, indp_d, out_d)
    nc.compile()
    return nc


def _emit(tc, xT_d, xT8_d, wqk8_d, wv_d, bqk_d, bvb_d, wout_d, boutb_d,
          trimask_d, indp_d, out_d):
    nc = tc.nc

    with (
        tc.tile_pool(name="consts", bufs=1) as consts,
        tc.tile_pool(name="dram", bufs=1, space="DRAM") as dram,  # noqa: F841
    ):
        # ---- constants resident for the whole kernel ----
        # (sixth-0 x inputs are hoisted in front of the big wout const so
        # the first projection matmuls aren't starved behind it)
        xT8_r = xT8_d.rearrange("(kc p) s -> p kc s", p=128)
        xT_r = xT_d.rearrange("(kc p) s -> p kc s", p=128)
        bqk_sb = consts.tile([128, 16], F32)
        nc.gpsimd.dma_start(bqk_sb[:], bqk_d[:])
        bvb_sb = consts.tile([128, D], FP16)
        nc.gpsimd.dma_start(bvb_sb[:], bvb_d.to_broadcast((128, D)))
        boutb_sb = consts.tile([128, D], FP16)
        nc.gpsimd.dma_start(boutb_sb[:], boutb_d.to_broadcast((128, D)))
        # band mask for halo chunks 0/2 only (center chunk is always valid),
        # duplicated over the head-pair dim so one op masks both heads
        trimask_sb = consts.tile([128, 2, 2, W], FP16)
        for h2 in range(2):
            nc.gpsimd.dma_start(
                trimask_sb[:, h2],
                trimask_d.rearrange("p (c w) -> p c w", c=2),
            )
        ind_sb = consts.tile([128, T_EXT], F32)
        nc.gpsimd.dma_start(ind_sb[:], indp_d.rearrange("t p -> p t"))
        ident_sb = consts.tile([128, 128], FP16)
        make_identity(nc, ident_sb[:])
        wout_sb = consts.tile([128, KC, D], FP16)

        # ---- persistent SBUF stores for q/k/v (no DRAM spill) ----
        with (
            tc.tile_pool(name="stores", bufs=1) as stores,
            tc.tile_pool(name="p2_e", bufs=4) as p2_e,
            tc.tile_pool(name="p2_sm", bufs=6) as p2_sm,
            tc.tile_pool(name="p2_ao", bufs=2) as p2_ao,
            tc.tile_pool(name="p2_out", bufs=2) as p2_out,
            tc.tile_pool(name="p1_ps", bufs=2, space="PSUM") as p1_ps,
            tc.tile_pool(name="ps_s", bufs=2, space="PSUM") as ps_s_pool,
            tc.tile_pool(name="ps_ot", bufs=2, space="PSUM") as ps_ot_pool,
            tc.tile_pool(name="p1_w", bufs=1) as p1_w,
            tc.tile_pool(name="p1_x", bufs=2) as p1_x,
        ):
            # q/k in (feature-pair partitions x positions); k in fp8 so the
            # score LDWEIGHTS runs at 4x FWL rate (the row-tiled score pairs
            # are LDW-bound otherwise); v in (positions x head x 65) with
            # the indicator column
            q_store = stores.tile([128, KC, S_EXT], FP16)
            k_store = stores.tile([128, KC, S_EXT], FP8)
            v_store = stores.tile([128, T_EXT, H, VCOL], FP16)

            # x8 is fully SBUF-resident, loaded per-kc so each DMA moves
            # contiguous 2304B rows, split across both HW DMA queues so the
            # first q/k m-block (which needs all 8 kc chunks) isn't gated on
            # one queue. (gpsimd DMAs ride the slow software DGE — avoid.)
            x8_sb = stores.tile([128, KC, S_EXT], FP8)
            for kc in range(4):
                nc.scalar.dma_start(
                    x8_sb[:, kc, :], xT8_d[kc * 128 : (kc + 1) * 128, :]
                )

            def load_x(sixth):
                tp0 = sixth * SIXTH
                xT_sb = p1_x.tile([128, KC, SIXTH], FP16, tag="xt", name="xT_sb")
                for h in range(2):
                    nc.scalar.dma_start(
                        xT_sb[:, 4 * h : 4 * h + 4, :],
                        xT_r[:, 4 * h : 4 * h + 4, tp0 : tp0 + SIXTH],
                    )
                return xT_sb

            x_hoisted = load_x(0)

            # weights: fp8 q/k + fp16 v weights on sync as per-(piece,kc)
            # contiguous-block DMAs (parallel across DMA engines); the first
            # matmuls only wait for w8 piece 0. wout (needed last) on gpsimd
            # after the sixth-0 x loads.
            w8_sb = p1_w.tile([128, KC, NQK], FP8)
            wv_sb = p1_w.tile([128, KC, D], FP16)

            def load_w8_piece(piece):
                for kc in range(KC):
                    r0 = (piece * KC + kc) * 128
                    nc.sync.dma_start(
                        w8_sb[:, kc, piece * 512 : (piece + 1) * 512],
                        wqk8_d[r0 : r0 + 128, :],
                    )

            def load_wv_piece(piece):
                for kc in range(KC):
                    r0 = (piece * KC + kc) * 128
                    nc.sync.dma_start(
                        wv_sb[:, kc, piece * 512 : (piece + 1) * 512],
                        wv_d[r0 : r0 + 128, :],
                    )

            load_w8_piece(0)
            for kc in range(4, KC):
                nc.sync.dma_start(
                    x8_sb[:, kc, :], xT8_d[kc * 128 : (kc + 1) * 128, :]
                )
            load_w8_piece(1)
            load_w8_piece(2)
            load_w8_piece(3)
            load_wv_piece(0)
            load_wv_piece(1)
            for kc in range(KC):
                nc.sync.dma_start(
                    wout_sb[:, kc, :], wout_d[kc * 128 : (kc + 1) * 128, :]
                )

            # ---- attention emission (software-pipelined per sixth) ----
            def emit_scores(e, a):
                """QK^T for both heads of pair a, + exp + band mask.
                Returns the masked exp tile."""
                # scoresT for both heads of the pair; the 4th c-slot is
                # padding so each head owns exactly one PSUM bank (the
                # two heads' matmuls run concurrently via row tiling)
                ps_s = ps_s_pool.tile([128, 2, 4, W], F32, tag="ps_s", name="ps_s")
                for c in range(3):
                    t = e - 1 + c
                    for h2 in range(2):
                        pr = slice(64 * h2, 64 * h2 + 64)
                        nc.tensor.matmul(
                            ps_s[:, h2, c, :],
                            lhsT=k_store[pr, a, t * 128 : (t + 1) * 128],
                            rhs=q_store[pr, a, e * 128 : (e + 1) * 128],
                            start=True,
                            stop=True,
                        )
                e_sb = p2_e.tile([128, 2, 3, W], FP16, name="e_sb")
                nc.scalar.activation(
                    e_sb[:],
                    ps_s[:, :, 0:3, :],
                    mybir.ActivationFunctionType.Exp,
                )
                # zero the out-of-band triangles of the halo chunks (0/2)
                nc.vector.tensor_mul(
                    e_sb[:, :, 0:3:2, :], e_sb[:, :, 0:3:2, :], trimask_sb[:]
                )
                return e_sb

            def emit_pv(e, a, e_sb):
                """e^T @ v for both heads (with indicator/denominator in
                column 64), then normalize. Returns the normalized tile."""
                ps_o = ps_ot_pool.tile([128, 512], F32, tag="ot", name="ps_o")
                for h2 in range(2):
                    for c in range(3):
                        nc.tensor.matmul(
                            ps_o[:, 65 * h2 : 65 * h2 + VCOL],
                            lhsT=e_sb[:, h2, c, :],
                            rhs=v_store[:, e - 1 + c, 2 * a + h2, :],
                            start=(c == 0),
                            stop=(c == 2),
                        )
                pv = ps_o[:, 0:130].rearrange("p (h v) -> p h v", h=2)
                rcp = p2_sm.tile([128, 2], F32, tag="rcp", name="rcp")
                nc.vector.reciprocal(rcp[:], pv[:, :, DH])
                no_sb = p2_sm.tile([128, 2, DH], FP16, tag="no", name="no_sb")
                nc.vector.tensor_mul(
                    no_sb[:],
                    pv[:, :, 0:DH],
                    rcp[:, :, None].broadcast_to((128, 2, DH)),
                )
                return no_sb

            def emit_transpose(a, no_sb, aoT_sb):
                """aoT = no^T via a normal matmul against the identity:
                full-rate, no transpose-mode switch. Emitted AFTER the next
                prefetched score block so its wait on the DVE norm overlaps
                independent PE work."""
                ps_t = ps_ot_pool.tile([128, 512], F32, tag="ot", name="ps_t")
                nc.tensor.matmul(
                    ps_t[:, :128],
                    lhsT=no_sb.rearrange("p a d -> p (a d)"),
                    rhs=ident_sb[:],
                    start=True,
                    stop=True,
                )
                # all copies on DVE: the ACT queue is nearly tile-bound by
                # the exp chain during attention-heavy stretches
                nc.vector.tensor_copy(aoT_sb[:, a, :], ps_t[:, :128])

            def emit_outproj(e, aoT_sb):
                for g in range(2):
                    ps_f = ps_ot_pool.tile([128, 512], F32, tag="ot", name="ps_f")
                    for a in range(HP):
                        nc.tensor.matmul(
                            ps_f[:],
                            lhsT=aoT_sb[:, a, :],
                            rhs=wout_sb[:, a, g * 512 : (g + 1) * 512],
                            start=(a == 0),
                            stop=(a == HP - 1),
                        )
                    fo = p2_out.tile([128, 512], FP16, tag="fo", name="fo")
                    nc.vector.tensor_add(
                        fo[:], ps_f[:], boutb_sb[:, g * 512 : (g + 1) * 512]
                    )
                    nc.sync.dma_start(
                        out_d[(e - 1) * 128 : e * 128, g * 512 : (g + 1) * 512],
                        fo[:],
                    )

            def emit_attention(tiles):
                """Pipelined attention over this sixth's ready query tiles:
                scores/exp/mask run 2 head-pairs ahead of PV so the PE never
                waits on the ACT/DVE softmax chain."""
                work = [(e, a) for e in tiles for a in range(HP)]
                pending = {}
                aoT = {}
                for e in tiles:
                    aoT[e] = p2_ao.tile([128, HP, 128], FP16, name="aoT")
                for i in range(min(2, len(work))):
                    pending[work[i]] = emit_scores(*work[i])
                for i, (e, a) in enumerate(work):
                    e_sb = pending.pop((e, a))
                    no_sb = emit_pv(e, a, e_sb)
                    if i + 2 < len(work):
                        pending[work[i + 2]] = emit_scores(*work[i + 2])
                    emit_transpose(a, no_sb, aoT[e])
                    if a == HP - 1:
                        emit_outproj(e, aoT.pop(e))

            def emit_v(sixth, tt, xT_sb):
                """v projection for tile t: fp16, out layout (positions x
                features), indicator in column 64."""
                t = sixth * (SIXTH // 128) + tt
                for g in range(2):
                    ps = p1_ps.tile([128, 512], F32, tag="ps1", name="ps")
                    for kc in range(KC):
                        nc.tensor.matmul(
                            ps[:],
                            lhsT=xT_sb[:, kc, tt * 128 : (tt + 1) * 128],
                            rhs=wv_sb[:, kc, g * 512 : (g + 1) * 512],
                            start=(kc == 0),
                            stop=(kc == KC - 1),
                        )
                    nc.vector.tensor_add(
                        v_store[:, t, g * 8 : (g + 1) * 8, 0:DH],
                        ps.rearrange("p (h d) -> p h d", d=DH),
                        bvb_sb[:, g * 512 : (g + 1) * 512].rearrange(
                            "p (h d) -> p h d", d=DH
                        ),
                    )
                nc.vector.memset(v_store[:, t, :, DH : DH + 1], 1.0)
                # zero v and indicator at out-of-sequence positions
                nc.vector.tensor_scalar_mul(
                    v_store[:, t], v_store[:, t], ind_sb[:, t : t + 1]
                )

            def emit_qk(sixth):
                """q/k: fp8 DoubleRow matmuls, out layout (feature-chunk
                partitions x positions), written straight into the
                persistent stores. q is not needed for the halo tiles
                (first/last 128 positions)."""
                tp0 = sixth * SIXTH
                for m in range(16):
                    dst = q_store if m < KC else k_store
                    mm = m if m < KC else m - KC
                    o0, on = 0, SIXTH
                    if m < KC and sixth == 0:
                        o0, on = 128, SIXTH - 128
                    elif m < KC and sixth == 5:
                        o0, on = 0, SIXTH - 128
                    ps = p1_ps.tile([128, 512], F32, tag="ps1", name="ps")
                    for k2 in range(KC2):
                        nc.tensor.matmul(
                            ps[:, :on],
                            lhsT=w8_sb[:, 2 * k2 : 2 * k2 + 2, m * 128 : (m + 1) * 128],
                            rhs=x8_sb[:, 2 * k2 : 2 * k2 + 2, tp0 + o0 : tp0 + o0 + on],
                            start=(k2 == 0),
                            stop=(k2 == KC2 - 1),
                            perf_mode=mybir.MatmulPerfMode.DoubleRow,
                        )
                    nc.scalar.activation(
                        dst[:, mm, tp0 + o0 : tp0 + o0 + on],
                        ps[:, :on],
                        mybir.ActivationFunctionType.Identity,
                        bias=bqk_sb[:, m : m + 1],
                        scale=Q_SCALE if m < KC else QK_DESCALE,
                    )

            # The q/k projection runs one sixth AHEAD of the v/attention
            # stream: it depends only on the resident x8 + w8, so the two
            # front-loaded sixths give the DMA queues a ~27us dense-compute
            # window to land wv/xT/wout before v/attention needs them.
            # v tiles stay interleaved with ready attention tiles so the PE
            # has projection work while the softmax chain (ACT/DVE) catches
            # up, and the last sixth's attention isn't all serialized at
            # the end.
            emit_qk(0)
            emit_qk(1)
            xT_cur = x_hoisted
            for sixth in range(6):
                xT_next = load_x(sixth + 1) if sixth < 5 else None
                if sixth == 0:
                    for tt in range(3):
                        emit_v(sixth, tt, xT_cur)
                    emit_attention([1])
                else:
                    for tt in range(3):
                        emit_v(sixth, tt, xT_cur)
                        emit_attention([3 * sixth - 1 + tt])
                if sixth + 2 <= 5:
                    emit_qk(sixth + 2)
                xT_cur = xT_next


_NC_CACHE = None


def _get_nc():
    global _NC_CACHE
    if _NC_CACHE is None:
        _NC_CACHE = _build_nc()
    return _NC_CACHE


def _host_inputs(x, Wqkv, bqkv, Wout, bout):
    """Build the 8 per-core input maps."""
    x = np.asarray(x, dtype=np.float32)
    Wqkv = np.asarray(Wqkv, dtype=np.float32)
    bqkv = np.asarray(bqkv, dtype=np.float32)
    Wout = np.asarray(Wout, dtype=np.float32)
    bout = np.asarray(bout, dtype=np.float32)

    scale = np.float32(1.0 / np.sqrt(D))
    bs = bqkv.copy()
    bs[:D] *= scale

    bqk = np.ascontiguousarray(bs[:NQK].reshape(16, 128).T)  # (128, 16)
    bvb = bs[NQK:].reshape(1, D)
    boutb = bout.reshape(1, D)

    # pre-shuffle weights into (piece, kc, 128, 512) contiguous DMA blocks
    wqk8 = np.clip(Wqkv[:, :NQK] * W_SCALE, -240.0, 240.0).astype(
        ml_dtypes.float8_e4m3
    )
    wqk8 = np.ascontiguousarray(
        wqk8.reshape(KC, 128, 4, 512).transpose(2, 0, 1, 3)
    ).reshape(4 * KC * 128, 512)
    wv = Wqkv[:, NQK:].astype(np.float16)
    wv = np.ascontiguousarray(
        wv.reshape(KC, 128, 2, 512).transpose(2, 0, 1, 3)
    ).reshape(2 * KC * 128, 512)

    # band mask for halo chunks 0/2 in (j_within_chunk, chunk, i) layout
    jc = np.arange(128)[:, None]
    i = np.arange(128)[None, :]
    tm = np.ones((128, 2, 128), dtype=np.float32)
    tm[:, 0] = (jc >= i).astype(np.float32)
    tm[:, 1] = (jc <= i).astype(np.float32)
    trimask = tm.reshape(128, 2 * W).astype(np.float16)

    in_maps = []
    for core in range(N_CORES):
        b, half = core // 2, core % 2
        s0 = half * S_LOC
        lo, hi = s0 - W, s0 + S_LOC + W
        xp = np.zeros((S_EXT, D), dtype=np.float32)
        src_lo, src_hi = max(lo, 0), min(hi, S)
        xp[src_lo - lo : src_hi - lo] = x[b, src_lo:src_hi]
        xT = np.ascontiguousarray(xp.T)
        xT8 = np.clip(xT * X_SCALE, -240.0, 240.0).astype(ml_dtypes.float8_e4m3)

        valid = np.ones(S_EXT, dtype=np.float32)
        if lo < 0:
            valid[: -lo] = 0.0
        if hi > S:
            valid[S - hi :] = 0.0
        indp = np.ascontiguousarray(valid.reshape(T_EXT, 128))

        in_maps.append(
            {
                "xT": xT.astype(np.float16),
                "xT8": xT8,
                "wqk8": wqk8,
                "wv": wv,
                "bqk": bqk,
                "bvb": bvb.astype(np.float16),
                "wout": Wout.astype(np.float16),
                "boutb": boutb.astype(np.float16),
                "trimask": trimask,
                "indp": indp,
            }
        )
    return in_maps


def kernel(x, Wqkv, bqkv, Wout, bout, _trace=False, _trace_cores=None):
    in_maps = _host_inputs(x, Wqkv, bqkv, Wout, bout)
    nc = _get_nc()
    res = run_bass_kernel_spmd(
        nc,
        in_maps,
        list(range(N_CORES)),
        trace=_trace,
        trace_cores=_trace_cores,
    )
    out = np.empty((B, S, D), dtype=np.float32)
    for core in range(N_CORES):
        b, half = core // 2, core % 2
        s0 = half * S_LOC
        out[b, s0 : s0 + S_LOC] = res.results[core]["out"]
    if _trace:
        return out, res
    return out
